# revision 20
# baseline (speedup 1.0000x reference)
"""Trainium2 Bass kernel for CLSProcess: diagonal linear recurrence
state_t = y_t * state_{t-1} + x_t * z_t over [B=8, T=4096, units=1024].

Sharding: batch across the 8 cores (one batch element per core).

Design (v7):
  - bf16 I/O: z host-cast to bf16, output written bf16 and host-upcast
    (halves HBM traffic both ways; 2e-2 gate, measured ~8e-3).
  - Host does layout + gate-vector prep only (all on the [T]-sized x/y
    gate vectors; the [T,U] bulk math stays on device):
      zt    [ng,128,G*U] bf16 - z regrouped so group DMAs are 2x1MB
      yz    [1,T] f32  - y with block-start entries zeroed (scan reset)
      xdiag [128,T] bf16 - I[s==t%128] * x_s: scan identity injection
             with x pre-folded, so one scan yields the matmul lhsT
             Mx[t,s] = x_s * prod_{r=s+1..t} y_r
      selm  [128,T] bf16 - the carry matrix sel[s,t] = I[s==127] *
             (p_t = prod_{r=t0..t} y_r): zeros + the per-block cumprod
             row at partition 127
  - STAGGERED INDEPENDENT CHAINS over block ranges (5,7,9,11 blocks):
    each chain starts from zero carry; the dropped cross-chain
    influence decays by a product of >=640 y's (~0 in f32) except in
    the chain's first block, which is computed raw and patched by a
    late correction (sel @ prev-chain-tail). Stagger makes each
    correction's input ready ~2 steps before its consumer chain ends,
    so corrections overlap the main loop instead of serializing the
    tail. Per step, blocks interleave across active chains so the
    tensor engine pipeline never sits behind one chain's carry stall
    and HAM stays warm.
  - per block, two column-chains (0:512 / 512:1024) in separate PSUM
    banks. Drains: chain A -> one scalar-engine op [512]; chain B ->
    scalar [0:192] + vector [192:512] (balances the two engines given
    the vector engine also owns the scans and corrections).
  - per-block 256KB output DMAs alternate sync/gpsimd issuers.
"""

import numpy as np
import ml_dtypes

import concourse.bacc as bacc
import concourse.bass as bass
import concourse.mybir as mybir
import concourse.tile as tile
from concourse.bass_utils import run_bass_kernel_spmd

B = 8
T = 4096
F = 1026
U = 1024
L = 128
G = 8            # blocks per z/out group (DMA layout unit)
NB = T // L      # 32 blocks
NG = NB // G     # 4 groups
GL = G * L       # 1024 scan columns per group
GU = G * U       # 8192 output columns per group
CHAINS = [(0, 5), (5, 12), (12, 21), (21, 32)]  # (start block, end block)
f32 = mybir.dt.float32
bf16 = mybir.dt.bfloat16
BF = ml_dtypes.bfloat16


def build_nc() -> bass.Bass:
    nc = bacc.Bacc()
    zt_d = nc.dram_tensor("zt", [NG, L, GU], bf16, kind="ExternalInput")
    yz_d = nc.dram_tensor("yz", [1, T], f32, kind="ExternalInput")
    xdiag_d = nc.dram_tensor("xdiag", [L, T], bf16, kind="ExternalInput")
    selm_d = nc.dram_tensor("selm", [L, T], bf16, kind="ExternalInput")
    out_d = nc.dram_tensor("out", [NG, L, GU], bf16, kind="ExternalOutput")

    warm_d = nc.inline_tensor(np.zeros((1, 8), dtype=np.float32), name="warm")

    mult = mybir.AluOpType.mult
    add = mybir.AluOpType.add

    with tile.TileContext(nc) as tc:
        with (
            tc.tile_pool(name="const", bufs=1) as constp,
            tc.tile_pool(name="zpool", bufs=NG) as zpool,
            tc.tile_pool(name="mtpool", bufs=NG) as mtpool,
            tc.tile_pool(name="otpool", bufs=NG) as otpool,
            tc.tile_pool(name="psA", bufs=NG, space="PSUM") as psA,
        ):
            # gpsimd warmup: dummy broadcast pulls its ~6us IRAM load
            # into the DMA preamble window
            warm = constp.tile([1, 8], f32, tag="warm")
            nc.sync.dma_start(warm[:], warm_d[:, :])
            warmbc = constp.tile([L, 8], f32, tag="warmbc")
            nc.gpsimd.partition_broadcast(warmbc[:], warm[0:1, :])

            yz = constp.tile([1, T], f32, tag="yz")
            nc.sync.dma_start(yz[:], yz_d[:, :])
            # z loads lead the sync queue (first group in 512KB slices so
            # block 0's rhs lands ASAP); xdiag/sel issue from the
            # otherwise-idle scalar queue in parallel
            zts = []
            for g in range(NG):
                ztile = zpool.tile([L, GU], bf16, tag="z")
                if g == 0:
                    for q in range(4):
                        nc.sync.dma_start(
                            ztile[:, q * GU // 4 : (q + 1) * GU // 4],
                            zt_d[g, :, q * GU // 4 : (q + 1) * GU // 4],
                        )
                else:
                    nc.sync.dma_start(ztile[:, : GU // 2], zt_d[g, :, : GU // 2])
                    nc.sync.dma_start(ztile[:, GU // 2 :], zt_d[g, :, GU // 2 :])
                zts.append(ztile)
            ybc = constp.tile([L, T], f32, tag="ybc")
            for g in range(NG):
                nc.gpsimd.partition_broadcast(
                    ybc[:, g * GL : (g + 1) * GL], yz[0:1, g * GL : (g + 1) * GL]
                )
            xdiag = constp.tile([L, T], bf16, tag="xdiag")
            sel = constp.tile([L, T], bf16, tag="sel")
            for g in range(NG):
                nc.scalar.dma_start(
                    xdiag[:, g * GL : (g + 1) * GL], xdiag_d[:, g * GL : (g + 1) * GL]
                )
                nc.scalar.dma_start(
                    sel[:, g * GL : (g + 1) * GL], selm_d[:, g * GL : (g + 1) * GL]
                )

            mts, ots = [], []
            for g in range(NG):
                # scan split in halves (block boundary => independent)
                mt = mtpool.tile([L, GL], bf16, tag="mt")
                h = GL // 2
                for c0 in (0, h):
                    nc.vector.tensor_tensor_scan(
                        mt[:, c0 : c0 + h],
                        ybc[:, g * GL + c0 : g * GL + c0 + h],
                        xdiag[:, g * GL + c0 : g * GL + c0 + h],
                        0.0,
                        mult,
                        add,
                    )
                mts.append(mt)
                ot = otpool.tile([L, GU], bf16, tag="ot")
                ots.append(ot)

            NCH = len(CHAINS)
            prev = [None] * NCH
            DS = 855  # ACT/DVE drain split point (balances engine busy)
            max_len = max(e - s for s, e in CHAINS)
            for i in range(max_len):
                act = [c for c, (s, e) in enumerate(CHAINS) if s + i < e]
                pos = {}
                # main matmuls for this step across active chains first...
                for c in act:
                    k = CHAINS[c][0] + i
                    g, j = k // G, k % G
                    po = psA.tile([L, U], f32, tag="po")
                    pos[c] = po
                    mtk = mts[g][:, j * L : (j + 1) * L]
                    zk = zts[g][:, j * U : (j + 1) * U]
                    nc.tensor.matmul(
                        po[:, 0:512], mtk, zk[:, 0:512], start=True, stop=(i == 0)
                    )
                    nc.tensor.matmul(
                        po[:, 512:U], mtk, zk[:, 512:U], start=True, stop=(i == 0)
                    )
                # ...then carry matmuls + drains in chain order
                for c in act:
                    k = CHAINS[c][0] + i
                    g, j = k // G, k % G
                    po = pos[c]
                    if i > 0:
                        selk = sel[:, k * L : (k + 1) * L]
                        pv = prev[c]
                        nc.tensor.matmul(
                            po[:, 0:512], selk, pv[:, 0:512], start=False, stop=True
                        )
                        nc.tensor.matmul(
                            po[:, 512:U], selk, pv[:, 512:U], start=False, stop=True
                        )
                    ot = ots[g]
                    c0 = j * U
                    nc.scalar.copy(ot[:, c0 : c0 + DS], po[:, 0:DS])
                    nc.vector.tensor_copy(ot[:, c0 + DS : c0 + U], po[:, DS:U])
                    prev[c] = ot[:, c0 : c0 + U]
                    # per-block 256KB output DMA (junction blocks are
                    # patched and written at the end)
                    if not (i == 0 and c > 0):
                        eng = nc.gpsimd if (k % 2 == 0) else nc.sync
                        eng.dma_start(out_d[g, :, c0 : c0 + U], ot[:, c0 : c0 + U])

            # late junction corrections: chain c's first block gains
            # sel @ (chain c-1 tail); exact up to prod-of->=640-y's ~ 0
            for c in range(1, NCH):
                k = CHAINS[c][0]
                g, j = k // G, k % G
                pc = psA.tile([L, U], f32, tag="po")
                selk = sel[:, k * L : (k + 1) * L]
                pv = prev[c - 1]
                nc.tensor.matmul(pc[:, 0:512], selk, pv[:, 0:512], start=True, stop=True)
                nc.tensor.matmul(pc[:, 512:U], selk, pv[:, 512:U], start=True, stop=True)
                ot = ots[g]
                c0 = j * U
                nc.vector.tensor_add(ot[:, c0 : c0 + U], pc[:], ot[:, c0 : c0 + U])
                nc.sync.dma_start(out_d[g, :, c0 : c0 + U], ot[:, c0 : c0 + U])
    nc.finalize()
    return nc


_NC = None


def _get_nc() -> bass.Bass:
    global _NC
    if _NC is None:
        _NC = build_nc()
    return _NC


def prep_in_maps(inp: np.ndarray) -> list[dict]:
    in_maps = []
    ar = np.arange(L)
    for b in range(B):
        x = inp[b, :, 0]
        y = inp[b, :, 1]
        z = inp[b, :, 2:]
        zt = np.ascontiguousarray(
            z.astype(BF).reshape(NG, G, L, U).transpose(0, 2, 1, 3).reshape(NG, L, GU)
        )
        yz = y.copy()
        yz[::L] = 0.0
        yz = np.ascontiguousarray(yz.reshape(1, T))
        xd = np.zeros((L, T), dtype=BF)
        xd[ar[:, None], ar[:, None] + L * np.arange(NB)[None, :]] = (
            x.reshape(NB, L).T.astype(BF)
        )
        selm = np.zeros((L, T), dtype=BF)
        selm[L - 1, :] = np.cumprod(y.reshape(NB, L), axis=1).astype(BF).reshape(T)
        in_maps.append({"zt": zt, "yz": yz, "xdiag": xd, "selm": selm})
    return in_maps


def unpack_out(results: list[dict]) -> np.ndarray:
    outs = []
    for b in range(B):
        o = results[b]["out"]  # [NG, L, GU] bf16
        o = (
            np.asarray(o)
            .reshape(NG, L, G, U)
            .transpose(0, 2, 1, 3)
            .reshape(T, U)
            .astype(np.float32)
        )
        outs.append(o)
    return np.stack(outs, axis=0)


def kernel(**inputs: np.ndarray) -> np.ndarray:
    inp = np.ascontiguousarray(inputs["inputs"], dtype=np.float32)
    assert inp.shape == (B, T, F), inp.shape
    nc = _get_nc()
    res = run_bass_kernel_spmd(nc, prep_in_maps(inp), core_ids=list(range(B)))
    return unpack_out(res.results)


# revision 23
# speedup vs baseline: 1.0912x; 1.0912x over previous
"""Trainium2 Bass kernel for CLSProcess: diagonal linear recurrence
state_t = y_t * state_{t-1} + x_t * z_t over [B=8, T=4096, units=1024].

Sharding: batch across the 8 cores (one batch element per core).

Design (v9):
  - bf16 I/O: z host-cast to bf16, output written bf16 and host-upcast
    (halves HBM traffic both ways; 2e-2 gate, measured ~8e-3).
  - Host does layout + gate-vector prep only (all on the [T]-sized x/y
    gate vectors; the [T,U] bulk math stays on device):
      zt    [ng,128,G*U] bf16 - z regrouped so group DMAs are 2x1MB
      yz    [1,T] f32  - y with block-start entries zeroed (scan reset)
      xdiag [128,T] bf16 - I[s==t%128] * x_s: scan identity injection
             with x pre-folded, so one scan yields the matmul lhsT
             Mx[t,s] = x_s * prod_{r=s+1..t} y_r
      prow  [1,T] bf16 - p_t = prod_{r=t0..t} y_r per block; DMA'd into
             partition 127 of a zeroed tile -> sel[s,t] = I[s==127] p_t
  - FOUR INDEPENDENT CHAINS, one per group of 8 blocks: each chain
    starts from zero carry; the dropped cross-chain influence decays by
    a product of >=1024 y's (0 in f32) except in the chain's first
    block, which is computed raw and patched at the end by a late
    correction (sel @ prev-chain-tail). Per step, blocks interleave
    across the 4 chains so the tensor engine pipeline never sits behind
    one chain's carry stall.
  - issue-queue layout: z loads lead the sync DMA queue; xdiag rides
    the scalar queue; output DMAs alternate gpsimd/sync. A burst of
    dummy matmuls during the DMA preamble pre-warms the PE HAM clock
    gate so real matmuls start at 2.4 GHz.
  - per block, two column-chains (0:512 / 512:1024) in separate PSUM
    banks; each drain is split scalar[416]+vector[96] (balances engine
    busy given the vector engine also owns the scans/corrections).
"""

import numpy as np
import ml_dtypes

import concourse.bacc as bacc
import concourse.bass as bass
import concourse.mybir as mybir
import concourse.tile as tile
from concourse.bass_utils import run_bass_kernel_spmd

B = 8
T = 4096
F = 1026
U = 1024
L = 128
G = 8            # blocks per group (= per chain)
NB = T // L      # 32 blocks
NG = NB // G     # 4 groups = 4 chains
GL = G * L       # 1024 scan columns per group
GU = G * U       # 8192 output columns per group
DS = 416         # ACT/DVE drain split point within each 512 half
f32 = mybir.dt.float32
bf16 = mybir.dt.bfloat16
BF = ml_dtypes.bfloat16


def build_nc() -> bass.Bass:
    nc = bacc.Bacc()
    zt_d = nc.dram_tensor("zt", [NG, L, GU], bf16, kind="ExternalInput")
    yz_d = nc.dram_tensor("yz", [1, T], f32, kind="ExternalInput")
    xdiag_d = nc.dram_tensor("xdiag", [L, T], bf16, kind="ExternalInput")
    prow_d = nc.dram_tensor("prow", [1, T], bf16, kind="ExternalInput")
    out_d = nc.dram_tensor("out", [NG, L, GU], bf16, kind="ExternalOutput")

    warm_d = nc.inline_tensor(np.zeros((L, 640), dtype=np.float32), name="warm")

    mult = mybir.AluOpType.mult
    add = mybir.AluOpType.add

    with tile.TileContext(nc) as tc:
        with (
            tc.tile_pool(name="const", bufs=1) as constp,
            tc.tile_pool(name="zpool", bufs=NG) as zpool,
            tc.tile_pool(name="mtpool", bufs=NG) as mtpool,
            tc.tile_pool(name="otpool", bufs=NG) as otpool,
            tc.tile_pool(name="psA", bufs=NG, space="PSUM") as psA,
            tc.tile_pool(name="psB", bufs=NG, space="PSUM") as psB,
        ):
            # ---- preamble: warmups + input DMA issue spread ----
            warm = constp.tile([L, 640], f32, tag="warm")
            nc.sync.dma_start(warm[:, 0:16], warm_d[:, 0:16])
            # gpsimd warmup: dummy broadcast pulls its ~6us IRAM load
            # into the DMA preamble window
            warmbc = constp.tile([L, 8], f32, tag="warmbc")
            nc.gpsimd.partition_broadcast(warmbc[:], warm[0:1, 0:8])

            yz = constp.tile([1, T], f32, tag="yz")
            nc.sync.dma_start(yz[:], yz_d[:, :])
            # z loads lead the sync queue
            zts = []
            for g in range(NG):
                ztile = zpool.tile([L, GU], bf16, tag="z")
                nc.sync.dma_start(ztile[:, : GU // 2], zt_d[g, :, : GU // 2])
                nc.sync.dma_start(ztile[:, GU // 2 :], zt_d[g, :, GU // 2 :])
                zts.append(ztile)
            # xdiag on the scalar queue, in parallel with the z issues
            xdiag = constp.tile([L, T], bf16, tag="xdiag")
            for g in range(NG):
                nc.scalar.dma_start(
                    xdiag[:, g * GL : (g + 1) * GL], xdiag_d[:, g * GL : (g + 1) * GL]
                )

            # PE warmup: ~5us of dummy matmuls so HAM reaches K=8/8
            # before the first real block
            warmb = constp.tile([L, 640], bf16, tag="warmb")
            nc.vector.tensor_copy(warmb[:, 0:16], warm[:, 0:16])
            wps = psA.tile([L, 512], f32, tag="poA")
            for _ in range(12):
                nc.tensor.matmul(
                    wps[:], warmb[:, 0:128], warmb[:, 128:640],
                    start=True, stop=True,
                )

            # carry matrix: sel[s,t] = I[s==127] * p_t (zeros + host row);
            # memset on the scalar engine after its xdiag issues
            sel = constp.tile([L, T], bf16, tag="sel")
            nc.any.memset(sel[:], 0.0)
            nc.sync.dma_start(sel[L - 1 : L, :], prow_d[0:1, :])

            # y broadcast, chunked per group so group 0's scan starts early
            ybc = constp.tile([L, T], f32, tag="ybc")
            for g in range(NG):
                nc.gpsimd.partition_broadcast(
                    ybc[:, g * GL : (g + 1) * GL], yz[0:1, g * GL : (g + 1) * GL]
                )

            mts, ots = [], []
            for g in range(NG):
                # scan split in halves (block boundary => independent)
                mt = mtpool.tile([L, GL], bf16, tag="mt")
                h = GL // 2
                for c0 in (0, h):
                    nc.vector.tensor_tensor_scan(
                        mt[:, c0 : c0 + h],
                        ybc[:, g * GL + c0 : g * GL + c0 + h],
                        xdiag[:, g * GL + c0 : g * GL + c0 + h],
                        0.0,
                        mult,
                        add,
                    )
                mts.append(mt)
                ot = otpool.tile([L, GU], bf16, tag="ot")
                ots.append(ot)

            prevA = [None] * NG
            prevB = [None] * NG
            for j in range(G):
                pos = {}
                # main matmuls for this step across the 4 chains first...
                for c in range(NG):
                    k = c * G + j
                    poA = psA.tile([L, 512], f32, tag="poA")
                    poB = psB.tile([L, 512], f32, tag="poB")
                    pos[c] = (poA, poB)
                    mtk = mts[c][:, j * L : (j + 1) * L]
                    zk = zts[c][:, j * U : (j + 1) * U]
                    nc.tensor.matmul(
                        poA[:], mtk, zk[:, 0:512], start=True, stop=(j == 0)
                    )
                    nc.tensor.matmul(
                        poB[:], mtk, zk[:, 512:1024], start=True, stop=(j == 0)
                    )
                # ...then carry matmuls + drains in chain order
                for c in range(NG):
                    k = c * G + j
                    poA, poB = pos[c]
                    if j > 0:
                        selk = sel[:, k * L : (k + 1) * L]
                        nc.tensor.matmul(poA[:], selk, prevA[c], start=False, stop=True)
                        nc.tensor.matmul(poB[:], selk, prevB[c], start=False, stop=True)
                    ot = ots[c]
                    c0 = j * U
                    nc.scalar.copy(ot[:, c0 : c0 + DS], poA[:, 0:DS])
                    nc.vector.tensor_copy(ot[:, c0 + DS : c0 + 512], poA[:, DS:512])
                    nc.scalar.copy(ot[:, c0 + 512 : c0 + 512 + DS], poB[:, 0:DS])
                    nc.vector.tensor_copy(
                        ot[:, c0 + 512 + DS : c0 + 1024], poB[:, DS:512]
                    )
                    prevA[c] = ot[:, c0 : c0 + 512]
                    prevB[c] = ot[:, c0 + 512 : c0 + 1024]
                    # per-block 256KB output DMA (junction blocks are
                    # patched and written at the end)
                    if not (j == 0 and c > 0):
                        eng = nc.gpsimd if (k % 2 == 0) else nc.sync
                        eng.dma_start(out_d[c, :, c0 : c0 + U], ot[:, c0 : c0 + U])

            # late junction corrections: chain c's first block gains
            # sel @ (chain c-1 tail); exact up to prod-of-1024-y's ~ 0
            for c in range(1, NG):
                k = c * G
                pcA = psA.tile([L, 512], f32, tag="poA")
                pcB = psB.tile([L, 512], f32, tag="poB")
                selk = sel[:, k * L : (k + 1) * L]
                nc.tensor.matmul(pcA[:], selk, prevA[c - 1], start=True, stop=True)
                nc.tensor.matmul(pcB[:], selk, prevB[c - 1], start=True, stop=True)
                ot = ots[c]
                nc.vector.tensor_add(ot[:, 0:512], pcA[:], ot[:, 0:512])
                nc.vector.tensor_add(ot[:, 512:1024], pcB[:], ot[:, 512:1024])
                nc.sync.dma_start(out_d[c, :, 0:U], ot[:, 0:U])
    nc.finalize()
    return nc


_NC = None


def _get_nc() -> bass.Bass:
    global _NC
    if _NC is None:
        _NC = build_nc()
    return _NC


def prep_in_maps(inp: np.ndarray) -> list[dict]:
    in_maps = []
    ar = np.arange(L)
    for b in range(B):
        x = inp[b, :, 0]
        y = inp[b, :, 1]
        z = inp[b, :, 2:]
        zt = np.ascontiguousarray(
            z.astype(BF).reshape(NG, G, L, U).transpose(0, 2, 1, 3).reshape(NG, L, GU)
        )
        yz = y.copy()
        yz[::L] = 0.0
        yz = np.ascontiguousarray(yz.reshape(1, T))
        xd = np.zeros((L, T), dtype=BF)
        xd[ar[:, None], ar[:, None] + L * np.arange(NB)[None, :]] = (
            x.reshape(NB, L).T.astype(BF)
        )
        prow = np.cumprod(y.reshape(NB, L), axis=1).astype(BF).reshape(1, T)
        in_maps.append({"zt": zt, "yz": yz, "xdiag": xd, "prow": prow})
    return in_maps


def unpack_out(results: list[dict]) -> np.ndarray:
    outs = []
    for b in range(B):
        o = results[b]["out"]  # [NG, L, GU] bf16
        o = (
            np.asarray(o)
            .reshape(NG, L, G, U)
            .transpose(0, 2, 1, 3)
            .reshape(T, U)
            .astype(np.float32)
        )
        outs.append(o)
    return np.stack(outs, axis=0)


def kernel(**inputs: np.ndarray) -> np.ndarray:
    inp = np.ascontiguousarray(inputs["inputs"], dtype=np.float32)
    assert inp.shape == (B, T, F), inp.shape
    nc = _get_nc()
    res = run_bass_kernel_spmd(nc, prep_in_maps(inp), core_ids=list(range(B)))
    return unpack_out(res.results)


# revision 25
# speedup vs baseline: 1.1008x; 1.0087x over previous
"""Trainium2 Bass kernel for CLSProcess: diagonal linear recurrence
state_t = y_t * state_{t-1} + x_t * z_t over [B=8, T=4096, units=1024].

Sharding: batch across the 8 cores (one batch element per core).

Design (v9):
  - bf16 I/O: z host-cast to bf16, output written bf16 and host-upcast
    (halves HBM traffic both ways; 2e-2 gate, measured ~8e-3).
  - Host does layout + gate-vector prep only (all on the [T]-sized x/y
    gate vectors; the [T,U] bulk math stays on device):
      zt    [ng,128,G*U] bf16 - z regrouped so group DMAs are 2x1MB
      yz    [1,T] f32  - y with block-start entries zeroed (scan reset)
      xdiag [128,T] bf16 - I[s==t%128] * x_s: scan identity injection
             with x pre-folded, so one scan yields the matmul lhsT
             Mx[t,s] = x_s * prod_{r=s+1..t} y_r
      prow  [1,T] bf16 - p_t = prod_{r=t0..t} y_r per block; DMA'd into
             partition 127 of a zeroed tile -> sel[s,t] = I[s==127] p_t
  - FOUR INDEPENDENT CHAINS, one per group of 8 blocks: each chain
    starts from zero carry; the dropped cross-chain influence decays by
    a product of >=1024 y's (0 in f32) except in the chain's first
    block, which is computed raw and patched at the end by a late
    correction (sel @ prev-chain-tail). Per step, blocks interleave
    across the 4 chains so the tensor engine pipeline never sits behind
    one chain's carry stall.
  - issue-queue layout: z loads lead the sync DMA queue; xdiag rides
    the scalar queue; output DMAs alternate gpsimd/sync. A burst of
    dummy matmuls during the DMA preamble pre-warms the PE HAM clock
    gate so real matmuls start at 2.4 GHz.
  - per block, two column-chains (0:512 / 512:1024) in separate PSUM
    banks; each drain is split scalar[416]+vector[96] (balances engine
    busy given the vector engine also owns the scans/corrections).
"""

import numpy as np
import ml_dtypes

import concourse.bacc as bacc
import concourse.bass as bass
import concourse.mybir as mybir
import concourse.tile as tile
from concourse.bass_utils import run_bass_kernel_spmd

B = 8
T = 4096
F = 1026
U = 1024
L = 128
G = 8            # blocks per group (= per chain)
NB = T // L      # 32 blocks
NG = NB // G     # 4 groups = 4 chains
GL = G * L       # 1024 scan columns per group
GU = G * U       # 8192 output columns per group
DS = 275         # ACT/DVE drain split point within each 512 half
CHAINS = [(0, 5), (5, 12), (12, 21), (21, 32)]  # (start, end) blocks
f32 = mybir.dt.float32
bf16 = mybir.dt.bfloat16
BF = ml_dtypes.bfloat16


def build_nc() -> bass.Bass:
    nc = bacc.Bacc()
    zt_d = nc.dram_tensor("zt", [NG, L, GU], bf16, kind="ExternalInput")
    yz_d = nc.dram_tensor("yz", [1, T], f32, kind="ExternalInput")
    xdiag_d = nc.dram_tensor("xdiag", [L, T], bf16, kind="ExternalInput")
    prow_d = nc.dram_tensor("prow", [1, T], bf16, kind="ExternalInput")
    out_d = nc.dram_tensor("out", [NG, L, GU], bf16, kind="ExternalOutput")

    warm_d = nc.inline_tensor(np.zeros((L, 640), dtype=np.float32), name="warm")

    mult = mybir.AluOpType.mult
    add = mybir.AluOpType.add

    with tile.TileContext(nc) as tc:
        with (
            tc.tile_pool(name="const", bufs=1) as constp,
            tc.tile_pool(name="zpool", bufs=NG) as zpool,
            tc.tile_pool(name="mtpool", bufs=NG) as mtpool,
            tc.tile_pool(name="otpool", bufs=NG) as otpool,
            tc.tile_pool(name="psA", bufs=NG, space="PSUM") as psA,
            tc.tile_pool(name="psB", bufs=NG, space="PSUM") as psB,
        ):
            # ---- preamble: warmups + input DMA issue spread ----
            warm = constp.tile([L, 640], f32, tag="warm")
            nc.sync.dma_start(warm[:, 0:16], warm_d[:, 0:16])
            # gpsimd warmup: dummy broadcast pulls its ~6us IRAM load
            # into the DMA preamble window
            warmbc = constp.tile([L, 8], f32, tag="warmbc")
            nc.gpsimd.partition_broadcast(warmbc[:], warm[0:1, 0:8])

            yz = constp.tile([1, T], f32, tag="yz")
            nc.sync.dma_start(yz[:], yz_d[:, :])
            # z loads lead the sync queue
            zts = []
            for g in range(NG):
                ztile = zpool.tile([L, GU], bf16, tag="z")
                nc.sync.dma_start(ztile[:, : GU // 2], zt_d[g, :, : GU // 2])
                nc.sync.dma_start(ztile[:, GU // 2 :], zt_d[g, :, GU // 2 :])
                zts.append(ztile)
            # xdiag on the scalar queue, in parallel with the z issues
            xdiag = constp.tile([L, T], bf16, tag="xdiag")
            for g in range(NG):
                nc.scalar.dma_start(
                    xdiag[:, g * GL : (g + 1) * GL], xdiag_d[:, g * GL : (g + 1) * GL]
                )

            # PE warmup: ~5us of dummy matmuls so HAM reaches K=8/8
            # before the first real block
            warmb = constp.tile([L, 640], bf16, tag="warmb")
            nc.vector.tensor_copy(warmb[:, 0:16], warm[:, 0:16])
            wps = psA.tile([L, 512], f32, tag="poA")
            for _ in range(12):
                nc.tensor.matmul(
                    wps[:], warmb[:, 0:128], warmb[:, 128:640],
                    start=True, stop=True,
                )

            # carry matrix: sel[s,t] = I[s==127] * p_t (zeros + host row);
            # memset on the scalar engine after its xdiag issues
            sel = constp.tile([L, T], bf16, tag="sel")
            nc.any.memset(sel[:], 0.0)
            nc.sync.dma_start(sel[L - 1 : L, :], prow_d[0:1, :])

            # y broadcast, chunked per group so group 0's scan starts early
            ybc = constp.tile([L, T], f32, tag="ybc")
            for g in range(NG):
                nc.gpsimd.partition_broadcast(
                    ybc[:, g * GL : (g + 1) * GL], yz[0:1, g * GL : (g + 1) * GL]
                )

            # scan halves, emitted in the order the staggered chains
            # need them: step-0 blocks are 0(g0h0), 5(g0h1), 12(g1h1),
            # 21(g2h1); the remaining halves follow
            mts = [mtpool.tile([L, GL], bf16, tag="mt", name=f"mt{g}") for g in range(NG)]
            h = GL // 2
            for g, c0 in [(0, 0), (0, h), (1, h), (2, h), (1, 0), (2, 0), (3, 0), (3, h)]:
                nc.vector.tensor_tensor_scan(
                    mts[g][:, c0 : c0 + h],
                    ybc[:, g * GL + c0 : g * GL + c0 + h],
                    xdiag[:, g * GL + c0 : g * GL + c0 + h],
                    0.0,
                    mult,
                    add,
                )
            ots = []
            for g in range(NG):
                ot = otpool.tile([L, GU], bf16, tag="ot")
                ots.append(ot)

            NCH = len(CHAINS)
            prevA = [None] * NCH
            prevB = [None] * NCH
            max_len = max(e - s for s, e in CHAINS)
            for i in range(max_len):
                act = [c for c, (s, e) in enumerate(CHAINS) if s + i < e]
                pos = {}
                # main matmuls for this step across active chains first...
                for c in act:
                    k = CHAINS[c][0] + i
                    g, j = k // G, k % G
                    poA = psA.tile([L, 512], f32, tag="poA")
                    poB = psB.tile([L, 512], f32, tag="poB")
                    pos[c] = (poA, poB)
                    first = i == 0
                    mtk = mts[g][:, j * L : (j + 1) * L]
                    zk = zts[g][:, j * U : (j + 1) * U]
                    nc.tensor.matmul(poA[:], mtk, zk[:, 0:512], start=True, stop=first)
                    nc.tensor.matmul(
                        poB[:], mtk, zk[:, 512:1024], start=True, stop=first
                    )
                # ...then carry matmuls + drains in chain order
                for c in act:
                    k = CHAINS[c][0] + i
                    g, j = k // G, k % G
                    poA, poB = pos[c]
                    if i > 0:
                        selk = sel[:, k * L : (k + 1) * L]
                        nc.tensor.matmul(poA[:], selk, prevA[c], start=False, stop=True)
                        nc.tensor.matmul(poB[:], selk, prevB[c], start=False, stop=True)
                    ot = ots[g]
                    c0 = j * U
                    nc.scalar.copy(ot[:, c0 : c0 + DS], poA[:, 0:DS])
                    nc.vector.tensor_copy(ot[:, c0 + DS : c0 + 512], poA[:, DS:512])
                    nc.scalar.copy(ot[:, c0 + 512 : c0 + 512 + DS], poB[:, 0:DS])
                    nc.vector.tensor_copy(
                        ot[:, c0 + 512 + DS : c0 + 1024], poB[:, DS:512]
                    )
                    prevA[c] = ot[:, c0 : c0 + 512]
                    prevB[c] = ot[:, c0 + 512 : c0 + 1024]
                    # per-block 256KB output DMA (junction blocks are
                    # patched and written at the end)
                    if not (i == 0 and c > 0):
                        eng = nc.gpsimd if (k % 2 == 0) else nc.sync
                        eng.dma_start(out_d[g, :, c0 : c0 + U], ot[:, c0 : c0 + U])

            # late junction corrections: chain c's first block gains
            # sel @ (chain c-1 tail); exact up to prod-of->=640-y's ~ 0.
            # With the stagger, chain c-1 finishes ~2 steps before chain
            # c, so these overlap the main loop.
            for c in range(1, NCH):
                k = CHAINS[c][0]
                g, j = k // G, k % G
                pcA = psA.tile([L, 512], f32, tag="poA")
                pcB = psB.tile([L, 512], f32, tag="poB")
                selk = sel[:, k * L : (k + 1) * L]
                nc.tensor.matmul(pcA[:], selk, prevA[c - 1], start=True, stop=True)
                nc.tensor.matmul(pcB[:], selk, prevB[c - 1], start=True, stop=True)
                ot = ots[g]
                c0 = j * U
                nc.vector.tensor_add(ot[:, c0 : c0 + 512], pcA[:], ot[:, c0 : c0 + 512])
                nc.vector.tensor_add(
                    ot[:, c0 + 512 : c0 + 1024], pcB[:], ot[:, c0 + 512 : c0 + 1024]
                )
                nc.sync.dma_start(out_d[g, :, c0 : c0 + U], ot[:, c0 : c0 + U])
    nc.finalize()
    return nc


_NC = None


def _get_nc() -> bass.Bass:
    global _NC
    if _NC is None:
        _NC = build_nc()
    return _NC


def prep_in_maps(inp: np.ndarray) -> list[dict]:
    in_maps = []
    ar = np.arange(L)
    for b in range(B):
        x = inp[b, :, 0]
        y = inp[b, :, 1]
        z = inp[b, :, 2:]
        zt = np.ascontiguousarray(
            z.astype(BF).reshape(NG, G, L, U).transpose(0, 2, 1, 3).reshape(NG, L, GU)
        )
        yz = y.copy()
        yz[::L] = 0.0
        yz = np.ascontiguousarray(yz.reshape(1, T))
        xd = np.zeros((L, T), dtype=BF)
        xd[ar[:, None], ar[:, None] + L * np.arange(NB)[None, :]] = (
            x.reshape(NB, L).T.astype(BF)
        )
        prow = np.cumprod(y.reshape(NB, L), axis=1).astype(BF).reshape(1, T)
        in_maps.append({"zt": zt, "yz": yz, "xdiag": xd, "prow": prow})
    return in_maps


def unpack_out(results: list[dict]) -> np.ndarray:
    outs = []
    for b in range(B):
        o = results[b]["out"]  # [NG, L, GU] bf16
        o = (
            np.asarray(o)
            .reshape(NG, L, G, U)
            .transpose(0, 2, 1, 3)
            .reshape(T, U)
            .astype(np.float32)
        )
        outs.append(o)
    return np.stack(outs, axis=0)


def kernel(**inputs: np.ndarray) -> np.ndarray:
    inp = np.ascontiguousarray(inputs["inputs"], dtype=np.float32)
    assert inp.shape == (B, T, F), inp.shape
    nc = _get_nc()
    res = run_bass_kernel_spmd(nc, prep_in_maps(inp), core_ids=list(range(B)))
    return unpack_out(res.results)
